# revision 9
# baseline (speedup 1.0000x reference)
"""BinaryLinear (sign(x) @ sign(W).T + bias) on 8 trn2 NeuronCores.

Reference semantics (fp32):
    bw = where(W > 0, 1, -1); bx = where(x > 0, 1, -1)
    y  = bx @ bw.T + bias          x:[B,IN] W:[OUT,IN] bias:[OUT] y:[B,OUT]

Sharding: 2D mesh 4x2 — 4 batch shards x 2 out-feature shards. Each core:
    x_s:[2048,4096] w_s:[2048,4096] bias_s:[2048] -> y_s:[2048,2048]

Per-core kernel pipeline (values are +-1 so bf16/fp8 are exact; PSUM
accumulates fp32, and |sum| <= 4096 < 2^24 so results are exact integers):
  A) binarize: ACT Sign fp32 -> bf16, DMA to DRAM scratch in k-chunked
     layout [K_TILES, M, 512] (contiguous rows for the xbar transpose)
  B) transpose: xbar dma_start_transpose per (k-chunk, m-chunk) -> bf16
     [128, 4, mc] in SBUF, cast to fp8e4 caches kxm=[128,32,2048] (bx.T)
     and kxn=[128,32,2048] (bw.T), both SBUF-resident (8 MiB each)
  C) matmul: composable_matmul_tile_kernel over the caches; fp8 DoubleRow
     (contraction 256/matmul); bias added during PSUM->SBUF eviction.
"""

import numpy as np

import concourse.bass as bass
import concourse.tile as tile
from concourse import bacc, mybir
from concourse.bass import ds, ts
from concourse.bass_utils import run_bass_kernel_spmd
from concourse.kernels.tile_matmul import (
    ShapeInfo,
    TileKxM,
    TileKxN,
    TileMxN,
    composable_matmul_tile_kernel,
)

P = 128
B, IN, OUT = 8192, 4096, 4096
MESH_B, MESH_O = 4, 2  # 4 batch shards x 2 out shards = 8 cores
BS, OS = B // MESH_B, OUT // MESH_O  # per-core shard: 2048, 2048

F32 = mybir.dt.float32
BF16 = mybir.dt.bfloat16
FP8 = mybir.dt.float8e4


def build_binary_linear(Bs: int, In: int, Os: int):
    """Build the per-core bass program for x:[Bs,In] w:[Os,In] bias:[1,Os]."""
    KCH = 512  # k-chunk (columns per xbar transpose, = K_TILE)
    K_TILES = In // KCH  # 8
    KSUB = KCH // P  # 4
    MC = min(512, Bs)  # m-chunk for phase B transposes
    AKH = min(2048, In)  # phase-A tile width (k) to bound SBUF

    nc = bacc.Bacc(None, target_bir_lowering=False, debug=False)
    x = nc.dram_tensor("x", [Bs, In], F32, kind="ExternalInput")
    w = nc.dram_tensor("w", [Os, In], F32, kind="ExternalInput")
    bias = nc.dram_tensor("bias", [1, Os], F32, kind="ExternalInput")
    y = nc.dram_tensor("y", [Bs, Os], F32, kind="ExternalOutput")

    with tile.TileContext(nc) as tc:
        with (
            tc.tile_pool(name="dram", bufs=1, space="DRAM") as dram,
            tc.tile_pool(name="const", bufs=1) as const,
            tc.tile_pool(name="cache", bufs=1) as cache,
            tc.tile_pool(name="a_in", bufs=2) as a_in,
            tc.tile_pool(name="a_out", bufs=2) as a_out,
            tc.tile_pool(name="b_tmp", bufs=3) as b_tmp,
        ):
            # bias broadcast [P, Os] (DMA replicates the single DRAM row)
            bias_sb = const.tile([P, Os], F32)
            nc.sync.dma_start(bias_sb[:], bias[0:1, :].to_broadcast((P, Os)))

            # fp8 transposed caches, SBUF-resident. Layout:
            # cache[p, s=2t+j, m] holds bin[m, k] for k = 256t + 2p + j.
            # Both caches use the same k permutation, so the contraction is
            # unchanged; the library's DR subtile pairs (s=2t, 2t+1) cover
            # k-block [256t, 256t+256) exactly.
            kxm_cache = cache.tile([P, In // P, Bs], FP8)  # bx.T
            kxn_cache = cache.tile([P, In // P, Os], FP8)  # bw.T

            # fp8 scratch in DRAM (natural layout)
            x_scr = dram.tile([Bs, In], FP8)
            w_scr = dram.tile([Os, In], FP8)

            # ---- Phase A: binarize to fp8 scratch -----------------------
            # (v is_gt 0) - 0.5 -> {+0.5, -0.5}; exact for v == 0 too
            # (reference maps 0 -> -1). The 2x scale per operand is undone
            # by the *4 in the PSUM eviction. DVE only — GpSimd ALU is ~25x
            # slower at elementwise (measured 987us for 8.4M elems).
            def binarize_chunk(src, scr, mc0):
                for m0 in range(mc0, mc0 + MC, P):
                    for k0 in range(0, In, AKH):
                        t_in = a_in.tile([P, AKH], F32, tag="a_in")
                        nc.sync.dma_start(t_in[:], src[ds(m0, P), ds(k0, AKH)])
                        t_bin = a_out.tile([P, AKH], FP8, tag="a_out")
                        nc.vector.tensor_scalar(
                            t_bin[:],
                            t_in[:],
                            0.0,
                            0.5,
                            mybir.AluOpType.is_gt,
                            mybir.AluOpType.subtract,
                        )
                        nc.scalar.dma_start(scr[ds(m0, P), ds(k0, AKH)], t_bin[:])

            # ---- Phase B: transpose the uint16 view (fp8 k-pairs) -------
            # One xbar op per (m-chunk, 256-wide k-block): tmp[p, m] u16
            # holds fp8 bytes (k=256t+2p, k+1); ACT deinterleaves into the
            # cache at s = 2t, 2t+1.
            def cache_chunk(scr, cch, mc0):
                scr16 = scr.bitcast(mybir.dt.uint16)  # [M, In//2]
                for t in range(In // 256):
                    tmp = b_tmp.tile([P, MC], mybir.dt.uint16, tag="b_tmp")
                    nc.sync.dma_start_transpose(
                        tmp[:], scr16[ds(mc0, MC), ds(t * P, P)]
                    )
                    pairs = tmp.bitcast(FP8).rearrange("p (m j) -> p j m", j=2)
                    nc.scalar.copy(cch[:, ds(2 * t, 2), ds(mc0, MC)], pairs)

            # interleave x/w chunk-wise so both caches fill progressively
            # and the matmul phase can start after the first chunks land
            assert Bs == Os
            for mc0 in range(0, Bs, MC):
                binarize_chunk(x, x_scr, mc0)
                cache_chunk(x_scr, kxm_cache, mc0)
                binarize_chunk(w, w_scr, mc0)
                cache_chunk(w_scr, kxn_cache, mc0)

            # ---- Phase C: fp8 DoubleRow matmul + bias -------------------
            def kxm_producer(nc_, md: TileKxM):
                return kxm_cache[
                    :, ts(md.k_tile_idx, md.k_subtiles), ts(md.m_tile_idx, md.m_tile)
                ]

            def kxn_producer(nc_, md: TileKxN):
                return kxn_cache[
                    :, ts(md.k_tile_idx, md.k_subtiles), ts(md.n_tile_idx, md.n_tile)
                ]

            y3 = y.rearrange("(po pi) f -> pi po f", pi=P)

            def bias_reducer(nc_, psum, sbuf, md: TileMxN):
                # operands are +-0.5, so psum = y_int / 4
                n0 = md.n_tile_idx * md.n_tile + md.n_subtile_idx * md.n_subtile
                nc_.vector.scalar_tensor_tensor(
                    out=sbuf[:, 0, :],
                    in0=psum[:, : md.n_slice_size],
                    scalar=4.0,
                    in1=bias_sb[:, ds(n0, md.n_slice_size)],
                    op0=mybir.AluOpType.mult,
                    op1=mybir.AluOpType.add,
                )

            def y_consumer(nc_, mxn_tile, md: TileMxN):
                nc_.scalar.dma_start(
                    y3[
                        :,
                        ts(md.m_tile_idx, md.m_subtiles),
                        ds(md.n_tile_idx * md.n_tile, md.n_slice_size),
                    ],
                    mxn_tile[:, :, : md.n_slice_size],
                )

            composable_matmul_tile_kernel(
                tc,
                kxm_shape=ShapeInfo(pdims=((P, In // P),), fdims=(Bs,)),
                kxn_shape=ShapeInfo(pdims=((P, In // P),), fdims=(Os,)),
                output_type=F32,
                kxm_producer=kxm_producer,
                kxn_producer=kxn_producer,
                mxn_consumer=y_consumer,
                mxn_subtile_reducer=bias_reducer,
                MATMUL_FREE_DIM=512,
                MAX_TILE_SIZE=512,
                MAX_K_TILE_SIZE=KCH,
                cache_tiles=False,
                temps_n_bufs=2,
                psum_n_bufs=2,
            )

    nc.compile()
    return nc


_NC_CACHE = {}


def _get_nc(Bs, In, Os):
    key = (Bs, In, Os)
    if key not in _NC_CACHE:
        _NC_CACHE[key] = build_binary_linear(Bs, In, Os)
    return _NC_CACHE[key]


def kernel(x: np.ndarray, weight: np.ndarray, bias: np.ndarray) -> np.ndarray:
    assert x.shape == (B, IN) and weight.shape == (OUT, IN) and bias.shape == (OUT,)
    nc = _get_nc(BS, IN, OS)

    in_maps = []
    for c in range(8):
        bi, oi = divmod(c, MESH_O)
        in_maps.append(
            {
                "x": np.ascontiguousarray(x[bi * BS : (bi + 1) * BS]),
                "w": np.ascontiguousarray(weight[oi * OS : (oi + 1) * OS]),
                "bias": np.ascontiguousarray(bias[oi * OS : (oi + 1) * OS])[None, :],
            }
        )

    r = run_bass_kernel_spmd(nc, in_maps, core_ids=list(range(8)))

    out = np.empty((B, OUT), dtype=np.float32)
    for c in range(8):
        bi, oi = divmod(c, MESH_O)
        out[bi * BS : (bi + 1) * BS, oi * OS : (oi + 1) * OS] = r.results[c]["y"]
    return out


# revision 10
# speedup vs baseline: 1.2617x; 1.2617x over previous
"""BinaryLinear (sign(x) @ sign(W).T + bias) on 8 trn2 NeuronCores.

Reference semantics (fp32):
    bw = where(W > 0, 1, -1); bx = where(x > 0, 1, -1)
    y  = bx @ bw.T + bias          x:[B,IN] W:[OUT,IN] bias:[OUT] y:[B,OUT]

Sharding: 2D mesh 4x2 — 4 batch shards x 2 out-feature shards. Each core:
    x_s:[2048,4096] w_s:[2048,4096] bias_s:[2048] -> y_s:[2048,2048]

Per-core pipeline (operands become +-0.5, exact in fp8; PSUM accumulates
fp32 so sums are exact; *4 at eviction restores the +-1 scale):
  A) binarize on DVE: (v is_gt 0) - 0.5 -> fp8 scratch in DRAM
  B) xbar dma_start_transpose of the scratch's uint16 view (fp8 k-pairs)
     -> ACT deinterleaves pairs into SBUF-resident fp8 caches laid out
     [128, s, m] where s=2t+j holds k = 256t + 2p + j. Both caches use
     the same k permutation so the contraction is unchanged.
  C) fp8 DoubleRow matmuls (contraction 256 per MM) from the caches;
     bias fused into the PSUM->SBUF eviction on DVE.

DMA ring discipline (avoids head-of-line stalls at the HWDGE sequencers):
  sync ring: fp32 input loads only (dependency-free stream)
  scalar ring: scratch writes -> xbar transposes (+ ACT deint copies)
  gpsimd SWDGE: y output writes, bias broadcast
"""

import numpy as np

import concourse.bass as bass
import concourse.tile as tile
from concourse import bacc, mybir
from concourse.bass import ds, ts
from concourse.bass_utils import run_bass_kernel_spmd

P = 128
B, IN, OUT = 8192, 4096, 4096
MESH_B, MESH_O = 4, 2  # 4 batch shards x 2 out shards = 8 cores
BS, OS = B // MESH_B, OUT // MESH_O  # per-core shard: 2048, 2048

F32 = mybir.dt.float32
FP8 = mybir.dt.float8e4
U16 = mybir.dt.uint16
DR = mybir.MatmulPerfMode.DoubleRow


def build_binary_linear(Bs: int, In: int, Os: int):
    """Build the per-core bass program for x:[Bs,In] w:[Os,In] bias:[1,Os]."""
    MC = min(512, Bs)  # production chunk (rows) = matmul tile size
    AKH = min(1024, In)  # phase-A tile width (k)
    MT = NT = MC  # matmul tile sizes
    TT = In // 256  # 256-wide contraction blocks (DoubleRow)
    n_mt, n_nt = Bs // MT, Os // NT

    nc = bacc.Bacc(None, target_bir_lowering=False, debug=False)
    x = nc.dram_tensor("x", [Bs, In], F32, kind="ExternalInput")
    w = nc.dram_tensor("w", [Os, In], F32, kind="ExternalInput")
    bias = nc.dram_tensor("bias", [1, Os], F32, kind="ExternalInput")
    y = nc.dram_tensor("y", [Bs, Os], F32, kind="ExternalOutput")

    with tile.TileContext(nc) as tc:
        with (
            tc.tile_pool(name="dram", bufs=1, space="DRAM") as dram,
            tc.tile_pool(name="const", bufs=1) as const,
            tc.tile_pool(name="cache", bufs=1) as cache,
            tc.tile_pool(name="a_in", bufs=4) as a_in,
            tc.tile_pool(name="a_out", bufs=4) as a_out,
            tc.tile_pool(name="b_tmp", bufs=6) as b_tmp,
            tc.tile_pool(name="outs", bufs=3) as outs,
            tc.tile_pool(name="psum", bufs=2, space="PSUM") as psum,
        ):
            bias_sb = const.tile([P, Os], F32)
            nc.gpsimd.dma_start(bias_sb[:], bias[0:1, :].to_broadcast((P, Os)))

            # fp8 transposed caches, SBUF-resident (8 MiB each)
            kxm_cache = cache.tile([P, In // P, Bs], FP8)  # bx.T
            kxn_cache = cache.tile([P, In // P, Os], FP8)  # bw.T

            x_scr = dram.tile([Bs, In], FP8)
            w_scr = dram.tile([Os, In], FP8)

            def binarize_chunk(src, scr, mc0):
                for m0 in range(mc0, mc0 + MC, P):
                    for k0 in range(0, In, AKH):
                        t_in = a_in.tile([P, AKH], F32, tag="a_in")
                        nc.sync.dma_start(t_in[:], src[ds(m0, P), ds(k0, AKH)])
                        t_bin = a_out.tile([P, AKH], FP8, tag="a_out")
                        nc.vector.tensor_scalar(
                            t_bin[:],
                            t_in[:],
                            0.0,
                            0.5,
                            mybir.AluOpType.is_gt,
                            mybir.AluOpType.subtract,
                        )
                        nc.scalar.dma_start(scr[ds(m0, P), ds(k0, AKH)], t_bin[:])

            def cache_chunk(scr, cch, mc0):
                scr16 = scr.bitcast(U16)  # [M, In//2]
                for t in range(TT):
                    tmp = b_tmp.tile([P, MC], U16, tag="b_tmp")
                    nc.sync.dma_start_transpose(
                        tmp[:], scr16[ds(mc0, MC), ds(t * P, P)]
                    )
                    pairs = tmp.bitcast(FP8).rearrange("p (m j) -> p j m", j=2)
                    nc.scalar.copy(cch[:, ds(2 * t, 2), ds(mc0, MC)], pairs)

            # production order: x0 w0 x1 w1 ... (chunk c: x_i at c=2i, w_j
            # at c=2j+1); matmul pairs are emitted by readiness below
            assert Bs == Os
            for mc0 in range(0, Bs, MC):
                binarize_chunk(x, x_scr, mc0)
                cache_chunk(x_scr, kxm_cache, mc0)
                binarize_chunk(w, w_scr, mc0)
                cache_chunk(w_scr, kxn_cache, mc0)

            # ---- Phase C: fp8 DoubleRow matmuls -------------------------
            y3 = y.rearrange("(po pi) f -> pi po f", pi=P)
            pairs_mn = sorted(
                ((i, j) for i in range(n_mt) for j in range(n_nt)),
                key=lambda p: (max(2 * p[0], 2 * p[1] + 1), p[0], p[1]),
            )
            for mi, nj in pairs_mn:
                out_t = outs.tile([P, MT // P, NT], F32, tag="out")
                for m2 in range(MT // P):
                    ps = psum.tile([P, NT], F32, tag=f"ps{m2}")
                    for t in range(TT):
                        nc.tensor.matmul(
                            ps[:],
                            kxm_cache[:, ds(2 * t, 2), ds(mi * MT + m2 * P, P)],
                            kxn_cache[:, ds(2 * t, 2), ds(nj * NT, NT)],
                            start=(t == 0),
                            stop=(t == TT - 1),
                            perf_mode=DR,
                        )
                    # psum = y_int/4 (operands are +-0.5): y = 4*psum + bias
                    nc.vector.scalar_tensor_tensor(
                        out=out_t[:, m2, :],
                        in0=ps[:],
                        scalar=4.0,
                        in1=bias_sb[:, ds(nj * NT, NT)],
                        op0=mybir.AluOpType.mult,
                        op1=mybir.AluOpType.add,
                    )
                nc.gpsimd.dma_start(
                    y3[:, ds(mi * (MT // P), MT // P), ds(nj * NT, NT)], out_t[:]
                )

    nc.compile()
    return nc


_NC_CACHE = {}


def _get_nc(Bs, In, Os):
    key = (Bs, In, Os)
    if key not in _NC_CACHE:
        _NC_CACHE[key] = build_binary_linear(Bs, In, Os)
    return _NC_CACHE[key]


def kernel(x: np.ndarray, weight: np.ndarray, bias: np.ndarray) -> np.ndarray:
    assert x.shape == (B, IN) and weight.shape == (OUT, IN) and bias.shape == (OUT,)
    nc = _get_nc(BS, IN, OS)

    in_maps = []
    for c in range(8):
        bi, oi = divmod(c, MESH_O)
        in_maps.append(
            {
                "x": np.ascontiguousarray(x[bi * BS : (bi + 1) * BS]),
                "w": np.ascontiguousarray(weight[oi * OS : (oi + 1) * OS]),
                "bias": np.ascontiguousarray(bias[oi * OS : (oi + 1) * OS])[None, :],
            }
        )

    r = run_bass_kernel_spmd(nc, in_maps, core_ids=list(range(8)))

    out = np.empty((B, OUT), dtype=np.float32)
    for c in range(8):
        bi, oi = divmod(c, MESH_O)
        out[bi * BS : (bi + 1) * BS, oi * OS : (oi + 1) * OS] = r.results[c]["y"]
    return out


# revision 13
# speedup vs baseline: 1.3511x; 1.0709x over previous
"""BinaryLinear (sign(x) @ sign(W).T + bias) on 8 trn2 NeuronCores.

Reference semantics (fp32):
    bw = where(W > 0, 1, -1); bx = where(x > 0, 1, -1)
    y  = bx @ bw.T + bias          x:[B,IN] W:[OUT,IN] bias:[OUT] y:[B,OUT]

Sharding: 2D mesh 4x2 — 4 batch shards x 2 out-feature shards. Each core:
    x_s:[2048,4096] w_s:[2048,4096] bias_s:[2048] -> y_s:[2048,2048]

Per-core pipeline (operands become +-0.5, exact in fp8; PSUM accumulates
fp32 so sums are exact; *4 at eviction restores the +-1 scale):
  A) binarize on DVE: (v is_gt 0) - 0.5 -> fp8 scratch in DRAM
  B) xbar dma_start_transpose of the scratch's uint16 view (fp8 k-pairs)
     -> ACT deinterleaves pairs into SBUF-resident fp8 caches laid out
     [128, s, m] where s=2t+j holds k = 256t + 2p + j. Both caches use
     the same k permutation so the contraction is unchanged.
  C) fp8 DoubleRow matmuls (contraction 256 per MM) from the caches;
     bias fused into the PSUM->SBUF eviction on DVE.

DMA ring discipline (avoids head-of-line stalls at the HWDGE sequencers):
  sync ring: fp32 input loads only (dependency-free stream)
  scalar ring: scratch writes -> xbar transposes (+ ACT deint copies)
  gpsimd SWDGE: y output writes, bias broadcast
"""

import numpy as np

import concourse.bass as bass
import concourse.tile as tile
from concourse import bacc, mybir
from concourse.bass import ds, ts
from concourse.bass_utils import run_bass_kernel_spmd

P = 128
B, IN, OUT = 8192, 4096, 4096
MESH_B, MESH_O = 4, 2  # 4 batch shards x 2 out shards = 8 cores
BS, OS = B // MESH_B, OUT // MESH_O  # per-core shard: 2048, 2048

F32 = mybir.dt.float32
FP8 = mybir.dt.float8e4
U16 = mybir.dt.uint16
DR = mybir.MatmulPerfMode.DoubleRow


def build_binary_linear(Bs: int, In: int, Os: int):
    """Build the per-core bass program for x:[Bs,In] w:[Os,In] bias:[1,Os]."""
    MC = min(512, Bs)  # production chunk (rows) = matmul tile size
    AKH = min(2048, In)  # phase-A tile width (k)
    MT = NT = MC  # matmul tile sizes
    TT = In // 256  # 256-wide contraction blocks (DoubleRow)
    n_mt, n_nt = Bs // MT, Os // NT

    nc = bacc.Bacc(None, target_bir_lowering=False, debug=False)
    x = nc.dram_tensor("x", [Bs, In], F32, kind="ExternalInput")
    w = nc.dram_tensor("w", [Os, In], F32, kind="ExternalInput")
    bias = nc.dram_tensor("bias", [1, Os], F32, kind="ExternalInput")
    y = nc.dram_tensor("y", [Bs, Os], F32, kind="ExternalOutput")

    with tile.TileContext(nc) as tc:
        with (
            tc.tile_pool(name="dram", bufs=1, space="DRAM") as dram,
            tc.tile_pool(name="const", bufs=1) as const,
            tc.tile_pool(name="cache", bufs=1) as cache,
            tc.tile_pool(name="a_in", bufs=2) as a_in,
            tc.tile_pool(name="a_out", bufs=3) as a_out,
            tc.tile_pool(name="b_tmp", bufs=6) as b_tmp,
            tc.tile_pool(name="outs", bufs=2) as outs,
            tc.tile_pool(name="psum", bufs=2, space="PSUM") as psum,
        ):
            bias_sb = const.tile([P, Os], F32)
            nc.gpsimd.dma_start(bias_sb[:], bias[0:1, :].to_broadcast((P, Os)))

            # fp8 transposed caches, SBUF-resident (8 MiB each)
            kxm_cache = cache.tile([P, In // P, Bs], FP8)  # bx.T
            kxn_cache = cache.tile([P, In // P, Os], FP8)  # bw.T

            x_scr = dram.tile([Bs, In], FP8)
            w_scr = dram.tile([Os, In], FP8)

            def binarize_chunk(src, scr, mc0):
                for m0 in range(mc0, mc0 + MC, P):
                    t_bin = a_out.tile([P, In], FP8, tag="a_out")
                    for k0 in range(0, In, AKH):
                        t_in = a_in.tile([P, AKH], F32, tag="a_in")
                        nc.sync.dma_start(t_in[:], src[ds(m0, P), ds(k0, AKH)])
                        nc.vector.tensor_scalar(
                            t_bin[:, ds(k0, AKH)],
                            t_in[:],
                            0.0,
                            0.5,
                            mybir.AluOpType.is_gt,
                            mybir.AluOpType.subtract,
                        )
                    nc.scalar.dma_start(scr[ds(m0, P), :], t_bin[:])

            def cache_chunk(scr, cch, mc0):
                scr16 = scr.bitcast(U16)  # [M, In//2]
                for t in range(TT):
                    tmp = b_tmp.tile([P, MC], U16, tag="b_tmp")
                    nc.scalar.dma_start_transpose(
                        tmp[:], scr16[ds(mc0, MC), ds(t * P, P)]
                    )
                    pairs = tmp.bitcast(FP8).rearrange("p (m j) -> p j m", j=2)
                    nc.scalar.copy(cch[:, ds(2 * t, 2), ds(mc0, MC)], pairs)

            # production order: x0 w0 x1 w1 ... (chunk c: x_i at c=2i, w_j
            # at c=2j+1); matmul pairs are emitted by readiness below
            assert Bs == Os
            for mc0 in range(0, Bs, MC):
                binarize_chunk(x, x_scr, mc0)
                cache_chunk(x_scr, kxm_cache, mc0)
                binarize_chunk(w, w_scr, mc0)
                cache_chunk(w_scr, kxn_cache, mc0)

            # ---- Phase C: fp8 DoubleRow matmuls -------------------------
            y3 = y.rearrange("(po pi) f -> pi po f", pi=P)
            pairs_mn = sorted(
                ((i, j) for i in range(n_mt) for j in range(n_nt)),
                key=lambda p: (max(2 * p[0], 2 * p[1] + 1), p[0], p[1]),
            )
            for mi, nj in pairs_mn:
                out_t = outs.tile([P, MT // P, NT], F32, tag="out")
                for m2 in range(MT // P):
                    ps = psum.tile([P, NT], F32, tag=f"ps{m2}")
                    for t in range(TT):
                        nc.tensor.matmul(
                            ps[:],
                            kxm_cache[:, ds(2 * t, 2), ds(mi * MT + m2 * P, P)],
                            kxn_cache[:, ds(2 * t, 2), ds(nj * NT, NT)],
                            start=(t == 0),
                            stop=(t == TT - 1),
                            perf_mode=DR,
                        )
                    # psum = y_int/4 (operands are +-0.5): y = 4*psum + bias
                    nc.vector.scalar_tensor_tensor(
                        out=out_t[:, m2, :],
                        in0=ps[:],
                        scalar=4.0,
                        in1=bias_sb[:, ds(nj * NT, NT)],
                        op0=mybir.AluOpType.mult,
                        op1=mybir.AluOpType.add,
                    )
                nc.gpsimd.dma_start(
                    y3[:, ds(mi * (MT // P), MT // P), ds(nj * NT, NT)], out_t[:]
                )

    nc.compile()
    return nc


_NC_CACHE = {}


def _get_nc(Bs, In, Os):
    key = (Bs, In, Os)
    if key not in _NC_CACHE:
        _NC_CACHE[key] = build_binary_linear(Bs, In, Os)
    return _NC_CACHE[key]


def kernel(x: np.ndarray, weight: np.ndarray, bias: np.ndarray) -> np.ndarray:
    assert x.shape == (B, IN) and weight.shape == (OUT, IN) and bias.shape == (OUT,)
    nc = _get_nc(BS, IN, OS)

    in_maps = []
    for c in range(8):
        bi, oi = divmod(c, MESH_O)
        in_maps.append(
            {
                "x": np.ascontiguousarray(x[bi * BS : (bi + 1) * BS]),
                "w": np.ascontiguousarray(weight[oi * OS : (oi + 1) * OS]),
                "bias": np.ascontiguousarray(bias[oi * OS : (oi + 1) * OS])[None, :],
            }
        )

    r = run_bass_kernel_spmd(nc, in_maps, core_ids=list(range(8)))

    out = np.empty((B, OUT), dtype=np.float32)
    for c in range(8):
        bi, oi = divmod(c, MESH_O)
        out[bi * BS : (bi + 1) * BS, oi * OS : (oi + 1) * OS] = r.results[c]["y"]
    return out


# revision 31
# speedup vs baseline: 1.6868x; 1.2484x over previous
"""BinaryLinear (sign(x) @ sign(W).T + bias) on 8 trn2 NeuronCores.

Reference semantics (fp32):
    bw = where(W > 0, 1, -1); bx = where(x > 0, 1, -1)
    y  = bx @ bw.T + bias          x:[B,IN] W:[OUT,IN] bias:[OUT] y:[B,OUT]

Sharding: 2D mesh 4x2 - 4 batch shards x 2 out-feature shards. Each core:
    x_s:[2048,4096] w_s:[2048,4096] bias_s:[2048] -> y_s:[2048,2048]

Per-core pipeline (operands become +-0.5, exact in fp8e4m3; PSUM
accumulates fp32 so sums are exact; *4 at eviction restores +-1 scale):
  A) binarize on DVE: (v is_gt 0) - 0.5 -> fp8 scratch in DRAM (maps
     v == 0 to -0.5, matching the reference's where(v > 0) semantics)
  B) xbar dma_start_transpose of the scratch's uint16 view moves fp8
     k-PAIRS directly into the SBUF-resident caches. Cache byte layout
     per partition p: [t, m, j] holding bin[m, k] for k = 256t + 2p + j.
     Both caches share this k permutation, so the contraction result is
     unchanged.
  C) fp8 DoubleRow matmuls (contraction 256/MM): lhsT/rhs APs pair
     t-blocks (t, t+1) as the Ko dim (stride 2M bytes, %16 ok) with the
     m/n dim read at stride 2 (the byte interleave); j in {0,1} selects
     the byte plane. 16 MMs of [256k x 128m] @ [256k x 512n] per psum
     tile; bias fused into PSUM eviction on DVE; matmul (m,n) pairs
     emitted in cache-readiness order.

DMA ring discipline (measured on HW):
  - ALL DMA transposes on one HWDGE ring (scalar). Splitting them
    across rings hit the xbar_mode DMATranspose/DMACopy HW bug ->
    nondeterministic cache corruption under multi-core load.
  - sync ring: fp32 input loads only (dependency-free stream; keeping
    dependent writes off this ring avoids head-of-line issue stalls).
  - gpsimd SWDGE: scratch writes, y writes, bias broadcast.
"""

import numpy as np

import concourse.bass as bass
import concourse.tile as tile
from concourse import bacc, mybir
from concourse.bass import ds, ts
from concourse.bass_utils import run_bass_kernel_spmd

P = 128
B, IN, OUT = 8192, 4096, 4096
MESH_B, MESH_O = 4, 2  # 4 batch shards x 2 out shards = 8 cores
BS, OS = B // MESH_B, OUT // MESH_O  # per-core shard: 2048, 2048

F32 = mybir.dt.float32
FP8 = mybir.dt.float8e4
U16 = mybir.dt.uint16
DR = mybir.MatmulPerfMode.DoubleRow


def build_binary_linear(Bs: int, In: int, Os: int):
    """Build the per-core bass program for x:[Bs,In] w:[Os,In] bias:[1,Os]."""
    MC = min(512, Bs)  # production chunk (rows) = matmul tile size
    AKH = min(2048, In)  # phase-A tile width (k)
    MT = NT = MC  # matmul tile sizes
    TT = In // 256  # 256-wide contraction blocks (DoubleRow)
    n_mt, n_nt = Bs // MT, Os // NT

    nc = bacc.Bacc(None, target_bir_lowering=False, debug=False)
    x = nc.dram_tensor("x", [Bs, In], F32, kind="ExternalInput")
    w = nc.dram_tensor("w", [Os, In], F32, kind="ExternalInput")
    bias = nc.dram_tensor("bias", [1, Os], F32, kind="ExternalInput")
    y = nc.dram_tensor("y", [Bs, Os], F32, kind="ExternalOutput")

    with tile.TileContext(nc) as tc:
        with (
            tc.tile_pool(name="dram", bufs=1, space="DRAM") as dram,
            tc.tile_pool(name="const", bufs=1) as const,
            tc.tile_pool(name="cache", bufs=1) as cache,
            tc.tile_pool(name="a_in", bufs=3) as a_in,
            tc.tile_pool(name="a_out", bufs=2) as a_out,
            tc.tile_pool(name="outs", bufs=6) as outs,
            tc.tile_pool(name="psum", bufs=2, space="PSUM") as psum,
        ):
            bias_sb = const.tile([P, Os], F32)
            nc.gpsimd.dma_start(bias_sb[:], bias[0:1, :].to_broadcast((P, Os)))

            # fp8 transposed caches, SBUF-resident (8 MiB each), written
            # directly by the u16 xbar transposes. Byte layout per
            # partition p: [t, m, j] where (t, m, j) holds bin[m, k] for
            # k = 256t + 2p + j (k-pairs byte-interleaved along m).
            kxm_cache = cache.tile([P, TT, Bs * 2], FP8)  # bx.T
            kxn_cache = cache.tile([P, TT, Os * 2], FP8)  # bw.T
            kxm4 = kxm_cache.rearrange("p t (m j) -> p t m j", j=2)
            kxn4 = kxn_cache.rearrange("p t (m j) -> p t m j", j=2)

            x_scr = dram.tile([Bs, In], FP8)
            w_scr = dram.tile([Os, In], FP8)

            def binarize_chunk(src, scr, mc0):
                for m0 in range(mc0, mc0 + MC, P):
                    t_bin = a_out.tile([P, In], FP8, tag="a_out")
                    for k0 in range(0, In, AKH):
                        t_in = a_in.tile([P, AKH], F32, tag="a_in")
                        nc.sync.dma_start(t_in[:], src[ds(m0, P), ds(k0, AKH)])
                        nc.vector.tensor_scalar(
                            t_bin[:, ds(k0, AKH)],
                            t_in[:],
                            0.0,
                            0.5,
                            mybir.AluOpType.is_gt,
                            mybir.AluOpType.subtract,
                        )
                    nc.gpsimd.dma_start(scr[ds(m0, P), :], t_bin[:])

            def cache_chunk(scr, cch, mc0):
                # all xbar transposes stay on ONE ring (scalar): mixing
                # DMATranspose with DMACopy across rings trips the HW
                # xbar_mode transition bug (nondeterministic corruption)
                scr16 = scr.bitcast(U16)  # [M, In//2]
                cch16 = cch.bitcast(U16)  # [P, TT, M]
                for t in range(TT):
                    nc.scalar.dma_start_transpose(
                        cch16[:, t, ds(mc0, MC)], scr16[ds(mc0, MC), ds(t * P, P)]
                    )

            # production order: x0 w0 x1 w1 ... (chunk c: x_i at c=2i, w_j
            # at c=2j+1); matmul pairs are emitted by readiness below
            assert Bs == Os
            for mc0 in range(0, Bs, MC):
                binarize_chunk(x, x_scr, mc0)
                cache_chunk(x_scr, kxm_cache, mc0)
                binarize_chunk(w, w_scr, mc0)
                cache_chunk(w_scr, kxn_cache, mc0)

            # ---- Phase C: fp8 DoubleRow matmuls -------------------------
            y3 = y.rearrange("(po pi) f -> pi po f", pi=P)
            pairs_mn = sorted(
                ((i, j) for i in range(n_mt) for j in range(n_nt)),
                key=lambda p: (max(2 * p[0], 2 * p[1] + 1), p[0], p[1]),
            )
            for mi, nj in pairs_mn:
                for m2 in range(MT // P):
                    ps = psum.tile([P, NT], F32, tag=f"ps{m2}")
                    for t0 in range(0, TT, 2):
                        for j in range(2):
                            nc.tensor.matmul(
                                ps[:],
                                kxm4[:, ds(t0, 2), ds(mi * MT + m2 * P, P), j],
                                kxn4[:, ds(t0, 2), ds(nj * NT, NT), j],
                                start=(t0 == 0 and j == 0),
                                stop=(t0 == TT - 2 and j == 1),
                                perf_mode=DR,
                            )
                    # psum = y_int/4 (operands are +-0.5): y = 4*psum + bias
                    out_t = outs.tile([P, NT], F32, tag="out")
                    nc.vector.scalar_tensor_tensor(
                        out=out_t[:],
                        in0=ps[:],
                        scalar=4.0,
                        in1=bias_sb[:, ds(nj * NT, NT)],
                        op0=mybir.AluOpType.mult,
                        op1=mybir.AluOpType.add,
                    )
                    nc.gpsimd.dma_start(
                        y3[:, mi * (MT // P) + m2, ds(nj * NT, NT)], out_t[:]
                    )

    nc.compile()
    return nc


_NC_CACHE = {}


def _get_nc(Bs, In, Os):
    key = (Bs, In, Os)
    if key not in _NC_CACHE:
        _NC_CACHE[key] = build_binary_linear(Bs, In, Os)
    return _NC_CACHE[key]


def kernel(x: np.ndarray, weight: np.ndarray, bias: np.ndarray) -> np.ndarray:
    assert x.shape == (B, IN) and weight.shape == (OUT, IN) and bias.shape == (OUT,)
    nc = _get_nc(BS, IN, OS)

    in_maps = []
    for c in range(8):
        bi, oi = divmod(c, MESH_O)
        in_maps.append(
            {
                "x": np.ascontiguousarray(x[bi * BS : (bi + 1) * BS]),
                "w": np.ascontiguousarray(weight[oi * OS : (oi + 1) * OS]),
                "bias": np.ascontiguousarray(bias[oi * OS : (oi + 1) * OS])[None, :],
            }
        )

    r = run_bass_kernel_spmd(nc, in_maps, core_ids=list(range(8)))

    out = np.empty((B, OUT), dtype=np.float32)
    for c in range(8):
        bi, oi = divmod(c, MESH_O)
        out[bi * BS : (bi + 1) * BS, oi * OS : (oi + 1) * OS] = r.results[c]["y"]
    return out


# revision 33
# speedup vs baseline: 1.9089x; 1.1317x over previous
"""BinaryLinear (sign(x) @ sign(W).T + bias) on 8 trn2 NeuronCores.

Reference semantics (fp32):
    bw = where(W > 0, 1, -1); bx = where(x > 0, 1, -1)
    y  = bx @ bw.T + bias          x:[B,IN] W:[OUT,IN] bias:[OUT] y:[B,OUT]

Sharding: 2D mesh 4x2 - 4 batch shards x 2 out-feature shards. Each core:
    x_s:[2048,4096] w_s:[2048,4096] bias_s:[2048] -> y_s:[2048,2048]

Per-core pipeline (operands become +-0.5, exact in fp8e4m3; PSUM
accumulates fp32 so sums are exact; *4 at eviction restores +-1 scale):
  A) binarize on DVE: (v is_gt 0) - 0.5 -> fp8 scratch in DRAM (maps
     v == 0 to -0.5, matching the reference's where(v > 0) semantics)
  B) xbar dma_start_transpose of the scratch's uint16 view moves fp8
     k-PAIRS directly into the SBUF-resident caches. Cache byte layout
     per partition p: [t, m, j] holding bin[m, k] for k = 256t + 2p + j.
     Both caches share this k permutation, so the contraction result is
     unchanged.
  C) fp8 DoubleRow matmuls (contraction 256/MM): lhsT/rhs APs pair
     t-blocks (t, t+1) as the Ko dim (stride 2M bytes, %16 ok) with the
     m/n dim read at stride 2 (the byte interleave); j in {0,1} selects
     the byte plane. 16 MMs of [256k x 128m] @ [256k x 512n] per psum
     tile; bias fused into PSUM eviction on DVE; matmul (m,n) pairs
     emitted in cache-readiness order.

DMA ring discipline (measured on HW):
  - ALL DMA transposes on one HWDGE ring (scalar). Splitting them
    across rings hit the xbar_mode DMATranspose/DMACopy HW bug ->
    nondeterministic cache corruption under multi-core load.
  - sync ring: fp32 input loads only (dependency-free stream; keeping
    dependent writes off this ring avoids head-of-line issue stalls).
  - gpsimd SWDGE: scratch writes, y writes, bias broadcast.
"""

import numpy as np

import concourse.bass as bass
import concourse.tile as tile
from concourse import bacc, mybir
from concourse.bass import ds, ts
from concourse.bass_utils import run_bass_kernel_spmd

P = 128
B, IN, OUT = 8192, 4096, 4096
MESH_B, MESH_O = 4, 2  # 4 batch shards x 2 out shards = 8 cores
BS, OS = B // MESH_B, OUT // MESH_O  # per-core shard: 2048, 2048

F32 = mybir.dt.float32
FP8 = mybir.dt.float8e4
U16 = mybir.dt.uint16
DR = mybir.MatmulPerfMode.DoubleRow


def build_binary_linear(Bs: int, In: int, Os: int):
    """Build the per-core bass program for x:[Bs,In] w:[Os,In] bias:[1,Os]."""
    MC = min(1024, Bs)  # production chunk (rows)
    AKH = min(2048, In)  # phase-A tile width (k)
    MT = NT = 512  # matmul tile sizes
    TT = In // 256  # 256-wide contraction blocks (DoubleRow)
    n_mt, n_nt = Bs // MT, Os // NT

    nc = bacc.Bacc(None, target_bir_lowering=False, debug=False)
    x = nc.dram_tensor("x", [Bs, In], F32, kind="ExternalInput")
    w = nc.dram_tensor("w", [Os, In], F32, kind="ExternalInput")
    bias = nc.dram_tensor("bias", [1, Os], F32, kind="ExternalInput")
    y = nc.dram_tensor("y", [Bs, Os], F32, kind="ExternalOutput")

    with tile.TileContext(nc) as tc:
        with (
            tc.tile_pool(name="dram", bufs=1, space="DRAM") as dram,
            tc.tile_pool(name="const", bufs=1) as const,
            tc.tile_pool(name="cache", bufs=1) as cache,
            tc.tile_pool(name="a_in", bufs=3) as a_in,
            tc.tile_pool(name="a_out", bufs=4) as a_out,
            tc.tile_pool(name="outs", bufs=6) as outs,
            tc.tile_pool(name="psum", bufs=2, space="PSUM") as psum,
        ):
            bias_sb = const.tile([P, Os], F32)
            nc.gpsimd.dma_start(bias_sb[:], bias[0:1, :].to_broadcast((P, Os)))

            # fp8 transposed caches, SBUF-resident (8 MiB each), written
            # directly by the u16 xbar transposes. Byte layout per
            # partition p: [t, m, j] where (t, m, j) holds bin[m, k] for
            # k = 256t + 2p + j (k-pairs byte-interleaved along m).
            kxm_cache = cache.tile([P, TT, Bs * 2], FP8)  # bx.T
            kxn_cache = cache.tile([P, TT, Os * 2], FP8)  # bw.T
            kxm4 = kxm_cache.rearrange("p t (m j) -> p t m j", j=2)
            kxn4 = kxn_cache.rearrange("p t (m j) -> p t m j", j=2)

            x_scr = dram.tile([Bs, In], FP8)
            w_scr = dram.tile([Os, In], FP8)

            def binarize_chunk(src, scr, mc0):
                for m0 in range(mc0, mc0 + MC, P):
                    t_bin = a_out.tile([P, In], FP8, tag="a_out")
                    for k0 in range(0, In, AKH):
                        t_in = a_in.tile([P, AKH], F32, tag="a_in")
                        nc.sync.dma_start(t_in[:], src[ds(m0, P), ds(k0, AKH)])
                        nc.vector.tensor_scalar(
                            t_bin[:, ds(k0, AKH)],
                            t_in[:],
                            0.0,
                            0.5,
                            mybir.AluOpType.is_gt,
                            mybir.AluOpType.subtract,
                        )
                    nc.gpsimd.dma_start(scr[ds(m0, P), :], t_bin[:])

            def cache_chunk(scr, cch, mc0):
                # all xbar transposes stay on ONE ring (scalar): mixing
                # DMATranspose with DMACopy across rings trips the HW
                # xbar_mode transition bug (nondeterministic corruption)
                scr16 = scr.bitcast(U16)  # [M, In//2]
                cch16 = cch.bitcast(U16)  # [P, TT, M]
                for t in range(TT):
                    nc.scalar.dma_start_transpose(
                        cch16[:, t, ds(mc0, MC)], scr16[ds(mc0, MC), ds(t * P, P)]
                    )

            # production order: x0 w0 x1 w1 ... (chunk c: x_i at c=2i, w_j
            # at c=2j+1); matmul pairs are emitted by readiness below
            assert Bs == Os
            for mc0 in range(0, Bs, MC):
                binarize_chunk(x, x_scr, mc0)
                cache_chunk(x_scr, kxm_cache, mc0)
                binarize_chunk(w, w_scr, mc0)
                cache_chunk(w_scr, kxn_cache, mc0)

            # ---- Phase C: fp8 DoubleRow matmuls -------------------------
            y3 = y.rearrange("(po pi) f -> pi po f", pi=P)
            mt_per_chunk = MC // MT
            pairs_mn = sorted(
                ((i, j) for i in range(n_mt) for j in range(n_nt)),
                key=lambda p: (
                    max(2 * (p[0] // mt_per_chunk), 2 * (p[1] // mt_per_chunk) + 1),
                    p[0],
                    p[1],
                ),
            )
            for mi, nj in pairs_mn:
                for m2 in range(MT // P):
                    ps = psum.tile([P, NT], F32, tag=f"ps{m2}")
                    for t0 in range(0, TT, 2):
                        for j in range(2):
                            nc.tensor.matmul(
                                ps[:],
                                kxm4[:, ds(t0, 2), ds(mi * MT + m2 * P, P), j],
                                kxn4[:, ds(t0, 2), ds(nj * NT, NT), j],
                                start=(t0 == 0 and j == 0),
                                stop=(t0 == TT - 2 and j == 1),
                                perf_mode=DR,
                            )
                    # psum = y_int/4 (operands are +-0.5): y = 4*psum + bias
                    out_t = outs.tile([P, NT], F32, tag="out")
                    nc.vector.scalar_tensor_tensor(
                        out=out_t[:],
                        in0=ps[:],
                        scalar=4.0,
                        in1=bias_sb[:, ds(nj * NT, NT)],
                        op0=mybir.AluOpType.mult,
                        op1=mybir.AluOpType.add,
                    )
                    nc.gpsimd.dma_start(
                        y3[:, mi * (MT // P) + m2, ds(nj * NT, NT)], out_t[:]
                    )

    nc.compile()
    return nc


_NC_CACHE = {}


def _get_nc(Bs, In, Os):
    key = (Bs, In, Os)
    if key not in _NC_CACHE:
        _NC_CACHE[key] = build_binary_linear(Bs, In, Os)
    return _NC_CACHE[key]


def kernel(x: np.ndarray, weight: np.ndarray, bias: np.ndarray) -> np.ndarray:
    assert x.shape == (B, IN) and weight.shape == (OUT, IN) and bias.shape == (OUT,)
    nc = _get_nc(BS, IN, OS)

    in_maps = []
    for c in range(8):
        bi, oi = divmod(c, MESH_O)
        in_maps.append(
            {
                "x": np.ascontiguousarray(x[bi * BS : (bi + 1) * BS]),
                "w": np.ascontiguousarray(weight[oi * OS : (oi + 1) * OS]),
                "bias": np.ascontiguousarray(bias[oi * OS : (oi + 1) * OS])[None, :],
            }
        )

    r = run_bass_kernel_spmd(nc, in_maps, core_ids=list(range(8)))

    out = np.empty((B, OUT), dtype=np.float32)
    for c in range(8):
        bi, oi = divmod(c, MESH_O)
        out[bi * BS : (bi + 1) * BS, oi * OS : (oi + 1) * OS] = r.results[c]["y"]
    return out
